# revision 8
# baseline (speedup 1.0000x reference)
"""Trainium2 8-core kernel for biased-attention with sigmoid gating.

Reference computation (per batch b):
  q = heads(q_x @ Wq) * C**-0.5 ; k = heads(kv_x @ Wk) ; v = heads(kv_x @ Wv)
  a = softmax(q k^T + bias1 + bias2, axis=-1)
  o = (a @ v) gated by sigmoid(q_x @ Wg + bg), then @ Wo + bo

Shapes: B=2, Q=K=2048, CQ=CK=CV=256, H=8, C=32, CO=256.

Sharding: 8 cores = 2 batches x 4 query-quarters (512 rows each). Each core
computes all 8 heads for its rows; no cross-core communication is needed.

The biases are host-cast to bf16 (the 2e-2 rel-err budget easily covers the
~0.2% this adds), so each core streams 2 x 16.8 MB, a ~95 us floor at the
~358 GB/s per-core HBM ceiling. The kernel keeps every engine under that
floor; the critical one is the PE, whose HAM clock gate throttles it to
1.2 GHz unless it is nearly saturated, so the structure maximizes PE duty:
  - scores are produced directly in the transposed [k, q] orientation so the
    8.4M-element score plane is never transposed on chip;
  - TWO heads are processed in interleaved fashion so one head's QK->exp->PV
    dependency stalls are filled by the other head's matmuls, and their QK
    stationaries sit in different 32-row PE strips (4 heads per 128-partition
    slot at bases 0/32/64/96), letting LDWEIGHTS overlap matmuls;
  - the score+bias add alternates between the PE (identity-matmul accumulate
    into the QK^T PSUM bank; exp reads PSUM) and the DVE (PSUM add at 1x),
    splitting that work across both engines;
  - b1+b2 presum runs on DVE at the bf16 2x rate (3 of 4 quarters) and on
    GpSimd (1 of 4);
  - all elementwise ops work on [128, 1024] double-k-tile slabs to amortize
    the TRN2 per-instruction fixed costs;
  - V carries an extra all-ones column per head so PV emits softmax
    denominators for free; a tiny [33,128] PE back-transpose restores natural
    orientation, where one fused scalar_tensor_tensor applies 1/denom and the
    sigmoid gate in a single DVE op;
  - K/Q projections use M=128 (4 heads per matmul); head-slot 1's projections
    are deferred into the main loop to cut the startup ramp.
"""

import numpy as np

B, Q, K, CQ, H, C, CO = 2, 2048, 2048, 256, 8, 32, 256
HC = H * C  # 256
QS = Q // 4  # 512 query rows per core
KT_N = K // 128  # 16 k-tiles
NUNIT = H * 4  # 32 (head, k-quarter) stream units
N_CORES = 8
SCALE = float(C) ** -0.5

_CACHED = {}


def _build():
    import concourse.bass as bass
    import concourse.mybir as mybir
    import concourse.tile as tile
    from concourse import bacc
    from concourse.masks import make_identity

    f32 = mybir.dt.float32
    bf16 = mybir.dt.bfloat16
    AF = mybir.ActivationFunctionType
    ALU = mybir.AluOpType

    nc = bacc.Bacc(None, target_bir_lowering=False)

    # activations arrive host-transposed and pre-cast to bf16: [C, rows]
    qxTd = nc.declare_dram_parameter("qxT", [CQ, QS], bf16, isOutput=False)
    kvxTd = nc.declare_dram_parameter("kvxT", [CQ, K], bf16, isOutput=False)
    # biases host-packed bf16 [H, 128p, 16kt*512q]: partition = k%128, free
    # dim runs over (k//128, q) so one head-quarter DMA reads 4 KB/partition
    b1d = nc.declare_dram_parameter("b1", [H, 128, KT_N * QS], bf16, isOutput=False)
    b2d = nc.declare_dram_parameter("b2", [H, 128, KT_N * QS], bf16, isOutput=False)
    # weights pre-cast to bf16 on host; Wq carries the C**-0.5 scale
    Wq = nc.declare_dram_parameter("Wq", [CQ, HC], bf16, isOutput=False)
    Wk = nc.declare_dram_parameter("Wk", [CQ, HC], bf16, isOutput=False)
    Wv = nc.declare_dram_parameter("Wv", [CQ, HC], bf16, isOutput=False)
    Wg = nc.declare_dram_parameter("Wg", [CQ, HC], bf16, isOutput=False)
    bg = nc.declare_dram_parameter("bg", [HC], f32, isOutput=False)
    Wo = nc.declare_dram_parameter("Wo", [HC, CO], bf16, isOutput=False)
    bo = nc.declare_dram_parameter("bo", [CO], f32, isOutput=False)
    out = nc.declare_dram_parameter("out", [QS, CO], f32, isOutput=True)

    with tile.TileContext(nc) as tc:
        with (
            tc.tile_pool(name="singles", bufs=1) as singles,
            tc.tile_pool(name="stage", bufs=2) as stage,
            tc.tile_pool(name="bias", bufs=1) as biasp,
            tc.tile_pool(name="work", bufs=2) as work,
            tc.tile_pool(name="ework", bufs=4) as ework,
            tc.tile_pool(name="ps", bufs=1, space="PSUM") as psp,
        ):
            # ---- setup loads; projection critical path (kvxT, Wk, Wq, qxT)
            # issued first on the scalar ring ----
            kvxT = singles.tile([128, 2, K], bf16, tag="kvxT")
            nc.scalar.dma_start(
                out=kvxT, in_=kvxTd[:, :].rearrange("(a p) k -> p a k", p=128)
            )
            wbf = {}
            for name, w in (("Wk", Wk), ("Wq", Wq)):
                wtile = singles.tile([128, 2, 256], bf16, tag=f"w_{name}")
                nc.scalar.dma_start(
                    out=wtile, in_=w[:, :].rearrange("(a p) c -> p a c", p=128)
                )
                wbf[name] = wtile
            qxT = singles.tile([128, 2, QS], bf16, tag="qxT")
            nc.scalar.dma_start(
                out=qxT, in_=qxTd[:, :].rearrange("(a p) q -> p a q", p=128)
            )
            for name, w in (("Wv", Wv), ("Wg", Wg), ("Wo", Wo)):
                wtile = singles.tile([128, 2, 256], bf16, tag=f"w_{name}")
                nc.scalar.dma_start(
                    out=wtile, in_=w[:, :].rearrange("(a p) c -> p a c", p=128)
                )
                wbf[name] = wtile
            bg_bc = singles.tile([128, HC], f32, tag="bg")
            nc.scalar.dma_start(out=bg_bc, in_=bg[:].partition_broadcast(128))
            bo_bc = singles.tile([128, CO], f32, tag="bo")
            nc.scalar.dma_start(out=bo_bc, in_=bo[:].partition_broadcast(128))

            # ---- bias streaming machinery (sync ring) ----
            # Stream unit si covers one (head, k-quarter); the stream order
            # interleaves the two heads of the active head-pair:
            #   si = hp*8 + qq*2 + hh  ->  head 2*hp+hh, k-quarter qq.
            def si_key(si):
                hp, rem = divmod(si, 8)
                qq, hh = divmod(rem, 2)
                return 2 * hp + hh, qq

            bias_tiles = {}

            def load_bias(si):
                h, qq = si_key(si)
                sl = slice(qq * 4 * QS, (qq + 1) * 4 * QS)
                t1 = biasp.tile([128, 4 * QS], bf16, tag="b1", bufs=7, name=f"b1_{si}")
                nc.sync.dma_start(out=t1, in_=b1d[h, :, sl])
                t2 = biasp.tile([128, 4 * QS], bf16, tag="b2", bufs=7, name=f"b2_{si}")
                nc.sync.dma_start(out=t2, in_=b2d[h, :, sl])
                bias_tiles[si] = (t1, t2)

            bsums = {}

            def presum(si):
                # b1+b2 for one quarter; mostly DVE (bf16 2x), 1-in-4 GpSimd
                t1, t2 = bias_tiles.pop(si)
                bs = biasp.tile([128, 4 * QS], bf16, tag="bs", bufs=4, name=f"bs_{si}")
                eng = nc.gpsimd if (si % 4 == 3) else nc.vector
                eng.tensor_tensor(bs, t1, t2, ALU.add)
                bsums[si] = bs

            LOOK = 6
            for si in range(LOOK):
                load_bias(si)

            ident = singles.tile([128, 128], bf16)
            make_identity(nc, ident)
            identf = singles.tile([128, 128], f32, tag="identf")
            make_identity(nc, identf)

            # Heads packed two per 128-partition tile at bases 0 and 32
            # (legal lhsT bases); head h lives at partitions (h%2)*32 of
            # pair slot h//2, so the two interleaved heads of a head-pair
            # occupy different 32-row PE strips.
            QT = singles.tile([128, H // 2, QS], bf16, tag="QT")
            KT = singles.tile([128, H // 2, K], bf16, tag="KT")

            def hsl(h):
                return slice((h % 2) * 32, (h % 2) * 32 + 32)

            def proj_pair(j):
                # K/Q projections for head-pair j; copies on ScalarE so the
                # DVE stays free for bias presums.
                for kc2 in range(2):
                    ps = psp.tile([128, 2 * QS, 1], f32, tag="scores", bufs=2)
                    for c in range(2):
                        for ck in range(2):
                            nc.tensor.matmul(
                                ps[:64, c * 512:(c + 1) * 512, 0],
                                wbf["Wk"][:, ck, j * 64:(j + 1) * 64],
                                kvxT[:, ck, (kc2 * 2 + c) * 512:(kc2 * 2 + c + 1) * 512],
                                start=(ck == 0),
                                stop=(ck == 1),
                            )
                    nc.scalar.copy(KT[:64, j, kc2 * 1024:(kc2 + 1) * 1024], ps[:64, :, 0])
                ps = psp.tile([128, 2 * QS, 1], f32, tag="scores", bufs=2)
                for ck in range(2):
                    nc.tensor.matmul(
                        ps[:64, :QS, 0],
                        wbf["Wq"][:, ck, j * 64:(j + 1) * 64],
                        qxT[:, ck, :],
                        start=(ck == 0),
                        stop=(ck == 1),
                    )
                nc.scalar.copy(QT[:64, j, :], ps[:64, :QS, 0])

            proj_pair(0)

            # V natural [128kr, 16kt, 8h*33] bf16; per head 32 V columns plus
            # an all-ones column so the PV matmul emits softmax denominators
            # for free in output column 32.
            Vn = singles.tile([128, KT_N, H * 33], bf16, tag="Vn")
            nc.gpsimd.memset(Vn, 1.0)
            for kt in range(KT_N):
                ps = psp.tile([128, 2 * QS, 1], f32, tag="scores", bufs=2)
                for ck in range(2):
                    nc.tensor.matmul(
                        ps[:, :HC, 0],
                        kvxT[:, ck, kt * 128:(kt + 1) * 128],
                        wbf["Wv"][:, ck, :],
                        start=(ck == 0),
                        stop=(ck == 1),
                    )
                nc.scalar.copy(
                    Vn[:, kt, :].rearrange("p (h x) -> p h x", x=33)[:, :, :32],
                    ps[:, :HC, 0].rearrange("p (h c) -> p h c", c=32),
                )

            # G natural [128q, 4qt, 256hc] f32 = sigmoid(qx @ Wg + bg),
            # computed up-front so the tail of the kernel stays short.
            Gn = singles.tile([128, 4, HC], f32, tag="Gn")
            for qt in range(4):
                ps = psp.tile([128, 2 * QS, 1], f32, tag="scores", bufs=2)
                for ck in range(2):
                    nc.tensor.matmul(
                        ps[:, :HC, 0],
                        qxT[:, ck, qt * 128:(qt + 1) * 128],
                        wbf["Wg"][:, ck, :],
                        start=(ck == 0),
                        stop=(ck == 1),
                    )
                gt = stage.tile([128, HC], f32, tag="gt")
                nc.vector.tensor_add(gt, ps[:, :HC, 0], bg_bc)
                nc.scalar.activation(Gn[:, qt, :], gt, AF.Sigmoid)

            presum(0)
            presum(1)

            # ---- main attention loop (transposed orientation) ----
            # Head-pairs are processed with their pair-units interleaved
            # (A0 B0 A1 B1 ...) so the PE always has an independent chain to
            # run while the other head waits on its exp/add.
            O_all = singles.tile([128, 4, HC], bf16, tag="O_all")
            pair_ctr = 0
            for hp in range(4):
                if hp < 3:
                    proj_pair(hp + 1)  # next pair's projections as PE filler
                o_ab = [
                    psp.tile([33, QS, 1], f32, tag="o_acc", bufs=2, name=f"oacc_{hp}_{x}")
                    for x in range(2)
                ]
                for qq in range(4):
                    for hh in range(2):
                        si = hp * 8 + qq * 2 + hh
                        if si + LOOK < NUNIT:
                            load_bias(si + LOOK)
                        if si + 2 < NUNIT:
                            presum(si + 2)
                    for half in range(2):
                        for hh in range(2):
                            h = 2 * hp + hh
                            si = hp * 8 + qq * 2 + hh
                            bs = bsums[si]
                            pe_path = pair_ctr % 2 == 0
                            pair_ctr += 1
                            s_ps = psp.tile([128, 2 * QS, 1], f32, tag="scores", bufs=2)
                            # dense high-activity matmul to keep the HAM
                            # clock gate at K=8/8 (the attention matmuls only
                            # toggle ~25% of the PE array, which reads as
                            # idle to the activity monitor and halves the PE
                            # clock); its output is overwritten by the QK
                            # start=True below.
                            nc.tensor.matmul(
                                s_ps[:, :QS, 0],
                                kvxT[:, 0, :128],
                                kvxT[:, 1, :QS],
                                start=True,
                                stop=True,
                                skip_group_check=True,
                            )
                            for j in range(2):
                                lkt = half * 2 + j
                                kt = qq * 4 + lkt
                                nc.tensor.matmul(
                                    s_ps[:, j * QS:(j + 1) * QS, 0],
                                    KT[hsl(h), h // 2, kt * 128:(kt + 1) * 128],
                                    QT[hsl(h), h // 2, :],
                                    start=True,
                                    stop=not pe_path,
                                )
                                if pe_path:
                                    nc.tensor.matmul(
                                        s_ps[:, j * QS:(j + 1) * QS, 0],
                                        ident,
                                        bs[:, lkt * QS:(lkt + 1) * QS],
                                        start=False,
                                        stop=True,
                                        skip_group_check=True,
                                    )
                            et = ework.tile([128, 2 * QS], bf16, tag="et", bufs=4)
                            if pe_path:
                                nc.scalar.activation(et, s_ps[:, :, 0], AF.Exp)
                            else:
                                tt = ework.tile([128, 2 * QS], bf16, tag="tt", bufs=3)
                                nc.vector.tensor_tensor(
                                    tt,
                                    s_ps[:, :, 0],
                                    bs[:, half * 2 * QS:(half * 2 + 2) * QS],
                                    ALU.add,
                                )
                                nc.scalar.activation(et, tt, AF.Exp)
                            for j in range(2):
                                kt = qq * 4 + half * 2 + j
                                nc.tensor.matmul(
                                    o_ab[hh][:, :, 0],
                                    Vn[:, kt, h * 33:(h + 1) * 33],
                                    et[:, j * QS:(j + 1) * QS],
                                    start=(kt == 0),
                                    stop=(kt == KT_N - 1),
                                )
                # per-head epilogue: back-transpose, normalize, gate
                for hh in range(2):
                    h = 2 * hp + hh
                    hcol = h * C
                    oT_sb = work.tile([33, QS], f32, tag="oT", bufs=2)
                    nc.scalar.copy(oT_sb, o_ab[hh][:, :, 0])
                    for qt in range(4):
                        on_ps = psp.tile([128, C + 1, 1], f32, tag="onat", bufs=1)
                        nc.tensor.transpose(
                            on_ps[:, :, 0],
                            oT_sb[:, qt * 128:(qt + 1) * 128],
                            identf[:C + 1, :C + 1],
                        )
                        rinv = work.tile([128, 1], f32, tag="rinv", bufs=2)
                        nc.vector.reciprocal(rinv, on_ps[:, C:C + 1, 0])
                        nc.vector.scalar_tensor_tensor(
                            O_all[:, qt, hcol:hcol + C],
                            on_ps[:, :C, 0],
                            rinv,
                            Gn[:, qt, hcol:hcol + C],
                            ALU.mult,
                            ALU.mult,
                        )

            # ---- output projection ----
            for qt in range(4):
                ogt_ps = psp.tile([128, 2, 128], bf16, tag="ogt", bufs=1)
                for hcc in range(2):
                    nc.tensor.transpose(
                        ogt_ps[:, hcc, :], O_all[:, qt, hcc * 128:(hcc + 1) * 128], ident
                    )
                ogt = stage.tile([128, 2, 128], bf16, tag="ogt_sb")
                nc.vector.tensor_copy(ogt, ogt_ps)
                f_ps = psp.tile([128, 2 * QS, 1], f32, tag="scores", bufs=2)
                for hcc in range(2):
                    nc.tensor.matmul(
                        f_ps[:, :CO, 0],
                        ogt[:, hcc, :],
                        wbf["Wo"][:, hcc, :],
                        start=(hcc == 0),
                        stop=(hcc == 1),
                    )
                o_sb = stage.tile([128, CO], f32, tag="o_out")
                nc.vector.tensor_add(o_sb, f_ps[:, :CO, 0], bo_bc)
                nc.sync.dma_start(out=out[qt * 128:(qt + 1) * 128, :], in_=o_sb)

    nc.compile()
    return nc


def _get_nc():
    if "nc" not in _CACHED:
        _CACHED["nc"] = _build()
    return _CACHED["nc"]


def kernel(**inputs):
    from concourse.bass_utils import run_bass_kernel_spmd

    import ml_dtypes

    bf = ml_dtypes.bfloat16
    nc = _get_nc()
    inp = {k: np.asarray(v, dtype=np.float32) for k, v in inputs.items()}
    wq_b = (inp["Wq"] * SCALE).astype(bf)
    wk_b = inp["Wk"].astype(bf)
    wv_b = inp["Wv"].astype(bf)
    wg_b = inp["Wg"].astype(bf)
    wo_b = inp["Wo"].astype(bf)

    def pack_bias(x, q0):
        # [H, Q, K] batch slice -> [H, 128p, 16kt*512q] bf16 with k = kt*128+p
        t = x[:, q0:q0 + QS, :].astype(bf)  # [H, QS, K]
        t = t.transpose(0, 2, 1)  # [H, K, QS]
        t = t.reshape(H, KT_N, 128, QS).transpose(0, 2, 1, 3)  # [H, p, kt, q]
        return np.ascontiguousarray(t).reshape(H, 128, KT_N * QS)

    in_maps = []
    for c in range(N_CORES):
        b, qi = c // 4, c % 4
        q0 = qi * QS
        in_maps.append({
            "qxT": np.ascontiguousarray(inp["q_x"][b, q0:q0 + QS, :].T).astype(bf),
            "kvxT": np.ascontiguousarray(inp["kv_x"][b].T).astype(bf),
            "b1": pack_bias(inp["bias1"][b], q0),
            "b2": pack_bias(inp["bias2"][b], q0),
            "Wq": wq_b, "Wk": wk_b, "Wv": wv_b, "Wg": wg_b,
            "bg": inp["bg"], "Wo": wo_b, "bo": inp["bo"],
        })
    res = run_bass_kernel_spmd(nc, in_maps, core_ids=list(range(N_CORES)))
    outa = np.empty((B, Q, CO), np.float32)
    for c in range(N_CORES):
        b, qi = c // 4, c % 4
        outa[b, qi * QS:(qi + 1) * QS, :] = res.results[c]["out"]
    return outa
